# revision 4
# baseline (speedup 1.0000x reference)
"""Haar wavelet (2x2 stride-2, per-channel) Trainium2 Bass kernel.

Full input x: (8, 64, 512, 512) f32 -> full output (8, 256, 256, 256) f32.
Sharding: pure data parallel over batch -- core i processes x[i].

Per-core layout (v2): partition = (channel c, row-half h) -> 128 partitions.
Block b covers input rows [b*4R, (b+1)*4R): partition (c,h) holds 2R
consecutive input rows of channel c -- ONE contiguous 8R KB DRAM run per
partition per load.  Output: partition (c,h) computes R output rows for
the 4 subband channels 4c..4c+3; each (q, R rows) is a contiguous 4R KB
DRAM run -> 4 runs per partition per store.  This keeps every DMA
descriptor >= 4KB (the SDMA packet size), which is what the 16 SDMA
engines are throughput-limited by: the previous layout's 1KB store
descriptors made the engines 98% busy on packet processing.

Engine roles:
  ACT   = load ring (HWDGE) + the x0.5 scaling (activation Copy, scale)
  DVE   = both butterflies (4+2 tensor_tensor per block, fp32 1x)
  SP    = store ring (HWDGE)
Moving the halve to ACT cuts DVE busy from ~360us to ~290us, below the
HBM roofline (~375us at 358 GB/s per core).
"""

import sys

if "/opt/trn_rl_repo" not in sys.path:
    sys.path.insert(0, "/opt/trn_rl_repo")

from contextlib import ExitStack

import numpy as np

import concourse.bass as bass
import concourse.tile as tile
from concourse import bacc
from concourse import mybir
from concourse.bass_utils import run_bass_kernel_spmd

N_CORES = 8
C, H, W = 64, 512, 512
F32 = mybir.dt.float32
ADD = mybir.AluOpType.add
SUB = mybir.AluOpType.subtract

_CACHED = {}


def _build(C=C, H=H, W=W, R=8):
    HO, WO = H // 2, W // 2
    NB = H // (4 * R)  # blocks; each covers 4R input rows across all channels
    nc = bacc.Bacc("TRN2", target_bir_lowering=False, debug=False)
    x = nc.dram_tensor("x", [C, H, W], F32, kind="ExternalInput").ap()
    out = nc.dram_tensor("out", [4 * C, HO, WO], F32, kind="ExternalOutput").ap()
    out4 = out.rearrange("(c q) ho w -> c q ho w", q=4)

    with tile.TileContext(nc) as tc, ExitStack() as ctx:
        xpool = ctx.enter_context(tc.tile_pool(name="xp", bufs=2))
        mpool = ctx.enter_context(tc.tile_pool(name="mid", bufs=2))
        rpool = ctx.enter_context(tc.tile_pool(name="res", bufs=2))

        def emit_load(b):
            # partition p = h*64 + c; h selects which 2R-row half of the
            # block's 4R input rows.  One DMA per half: single partition
            # dim (c), one contiguous 8R KB run per partition.
            xt = xpool.tile([128, 2 * R * W], F32)
            for h in (0, 1):
                src = x[
                    :, b * 4 * R + h * 2 * R : b * 4 * R + (h + 1) * 2 * R, :
                ].rearrange("c t w -> c (t w)")
                nc.scalar.dma_start(xt[h * 64 : (h + 1) * 64, :], src)
            return xt

        def emit_stage1(xt):
            # x0.5 on ACT (activation Copy with scale), in place
            nc.scalar.mul(xt[:], xt[:], 0.5)
            x4 = xt[:].rearrange("p (r t w) -> p r t w", t=2, w=W)
            top, bot = x4[:, :, 0, :], x4[:, :, 1, :]
            s_t = mpool.tile([128, R * W], F32, tag="s")
            d_t = mpool.tile([128, R * W], F32, tag="d")
            sv = s_t[:].rearrange("p (r w) -> p r w", w=W)
            dv = d_t[:].rearrange("p (r w) -> p r w", w=W)
            nc.vector.tensor_tensor(sv, top, bot, ADD)
            nc.vector.tensor_tensor(dv, bot, top, SUB)
            return s_t, d_t

        def emit_stage2(b, s_t, d_t):
            s2 = s_t[:].rearrange("p (r j t) -> p r j t", t=2, j=WO)
            d2 = d_t[:].rearrange("p (r j t) -> p r j t", t=2, j=WO)
            s_e, s_o = s2[:, :, :, 0], s2[:, :, :, 1]
            d_e, d_o = d2[:, :, :, 0], d2[:, :, :, 1]
            rt = rpool.tile([128, 4 * R * WO], F32)
            r4 = rt[:].rearrange("p (q r j) -> p q r j", q=4, j=WO)
            nc.vector.tensor_tensor(r4[:, 0], s_e, s_o, ADD)  # ll
            nc.vector.tensor_tensor(r4[:, 1], d_e, d_o, ADD)  # lh
            nc.vector.tensor_tensor(r4[:, 2], s_o, s_e, SUB)  # hl
            nc.vector.tensor_tensor(r4[:, 3], d_o, d_e, SUB)  # hh
            for h in (0, 1):
                dst = out4[:, :, b * 2 * R + h * R : b * 2 * R + (h + 1) * R, :]
                nc.sync.dma_start(dst, r4[h * 64 : (h + 1) * 64])

        xts = {0: emit_load(0)}
        pending = None  # (b, s_t, d_t)
        for b in range(NB):
            if b + 1 < NB:
                xts[b + 1] = emit_load(b + 1)
            s_t, d_t = emit_stage1(xts.pop(b))
            if pending is not None:
                emit_stage2(*pending)
            pending = (b, s_t, d_t)
        emit_stage2(*pending)
    nc.compile()
    return nc


def _get_nc():
    if "nc" not in _CACHED:
        _CACHED["nc"] = _build()
    return _CACHED["nc"]


def _run(x, **kwargs):
    x = np.ascontiguousarray(np.asarray(x), dtype=np.float32)
    assert x.shape == (N_CORES, C, H, W), x.shape
    nc = _get_nc()
    in_maps = [{"x": np.ascontiguousarray(x[i])} for i in range(N_CORES)]
    res = run_bass_kernel_spmd(nc, in_maps, core_ids=list(range(N_CORES)), **kwargs)
    out = np.stack([res.results[i]["out"] for i in range(N_CORES)], axis=0)
    return out, res


def kernel(x):
    return _run(x)[0]


# revision 5
# speedup vs baseline: 1.0847x; 1.0847x over previous
"""Haar wavelet (2x2 stride-2, per-channel) Trainium2 Bass kernel.

Full input x: (8, 64, 512, 512) f32 -> full output (8, 256, 256, 256) f32.
Sharding: pure data parallel over batch -- core i processes x[i].

Per-core layout (v2): partition = (channel c, row-half h) -> 128 partitions.
Block b covers input rows [b*4R, (b+1)*4R): partition (c,h) holds 2R
consecutive input rows of channel c -- ONE contiguous 8R KB DRAM run per
partition per load.  Output: partition (c,h) computes R output rows for
the 4 subband channels 4c..4c+3; each (q, R rows) is a contiguous 4R KB
DRAM run -> 4 runs per partition per store.  This keeps every DMA
descriptor >= 4KB (the SDMA packet size), which is what the 16 SDMA
engines are throughput-limited by: the previous layout's 1KB store
descriptors made the engines 98% busy on packet processing.

Engine roles:
  ACT   = load ring (HWDGE) + the x0.5 scaling (activation Copy, scale)
  DVE   = both butterflies (4+2 tensor_tensor per block, fp32 1x)
  SP    = store ring (HWDGE)
Moving the halve to ACT cuts DVE busy from ~360us to ~290us, below the
HBM roofline (~375us at 358 GB/s per core).
"""

import sys

if "/opt/trn_rl_repo" not in sys.path:
    sys.path.insert(0, "/opt/trn_rl_repo")

from contextlib import ExitStack

import numpy as np

import concourse.bass as bass
import concourse.tile as tile
from concourse import bacc
from concourse import mybir
from concourse.bass_utils import run_bass_kernel_spmd

N_CORES = 8
C, H, W = 64, 512, 512
F32 = mybir.dt.float32
ADD = mybir.AluOpType.add
SUB = mybir.AluOpType.subtract

_CACHED = {}


def _build(C=C, H=H, W=W, R=8):
    HO, WO = H // 2, W // 2
    NB = H // (4 * R)  # blocks; each covers 4R input rows across all channels
    nc = bacc.Bacc("TRN2", target_bir_lowering=False, debug=False)
    x = nc.dram_tensor("x", [C, H, W], F32, kind="ExternalInput").ap()
    out = nc.dram_tensor("out", [4 * C, HO, WO], F32, kind="ExternalOutput").ap()
    out4 = out.rearrange("(c q) ho w -> c q ho w", q=4)

    with tile.TileContext(nc) as tc, ExitStack() as ctx:
        xpool = ctx.enter_context(tc.tile_pool(name="xp", bufs=2))
        mpool = ctx.enter_context(tc.tile_pool(name="mid", bufs=2))
        rpool = ctx.enter_context(tc.tile_pool(name="res", bufs=2))

        def emit_load(b):
            # partition p = h*64 + c; h selects which 2R-row half of the
            # block's 4R input rows.  One DMA per half: single partition
            # dim (c), one contiguous 8R KB run per partition.
            xt = xpool.tile([128, 2 * R * W], F32)
            for h in (0, 1):
                src = x[
                    :, b * 4 * R + h * 2 * R : b * 4 * R + (h + 1) * 2 * R, :
                ].rearrange("c t w -> c (t w)")
                nc.sync.dma_start(xt[h * 64 : (h + 1) * 64, :], src)
            return xt

        def emit_stage1(xt):
            # x0.5 on ACT (activation Copy with scale), in place
            nc.scalar.mul(xt[:], xt[:], 0.5)
            x4 = xt[:].rearrange("p (r t w) -> p r t w", t=2, w=W)
            top, bot = x4[:, :, 0, :], x4[:, :, 1, :]
            s_t = mpool.tile([128, R * W], F32, tag="s")
            d_t = mpool.tile([128, R * W], F32, tag="d")
            sv = s_t[:].rearrange("p (r w) -> p r w", w=W)
            dv = d_t[:].rearrange("p (r w) -> p r w", w=W)
            nc.vector.tensor_tensor(sv, top, bot, ADD)
            nc.vector.tensor_tensor(dv, bot, top, SUB)
            return s_t, d_t

        def emit_stage2(b, s_t, d_t):
            s2 = s_t[:].rearrange("p (r j t) -> p r j t", t=2, j=WO)
            d2 = d_t[:].rearrange("p (r j t) -> p r j t", t=2, j=WO)
            s_e, s_o = s2[:, :, :, 0], s2[:, :, :, 1]
            d_e, d_o = d2[:, :, :, 0], d2[:, :, :, 1]
            rt = rpool.tile([128, 4 * R * WO], F32)
            r4 = rt[:].rearrange("p (q r j) -> p q r j", q=4, j=WO)
            nc.vector.tensor_tensor(r4[:, 0], s_e, s_o, ADD)  # ll
            nc.vector.tensor_tensor(r4[:, 1], d_e, d_o, ADD)  # lh
            nc.vector.tensor_tensor(r4[:, 2], s_o, s_e, SUB)  # hl
            nc.vector.tensor_tensor(r4[:, 3], d_o, d_e, SUB)  # hh
            for h in (0, 1):
                dst = out4[:, :, b * 2 * R + h * R : b * 2 * R + (h + 1) * R, :]
                nc.sync.dma_start(dst, r4[h * 64 : (h + 1) * 64])

        xts = {0: emit_load(0)}
        pending = None  # (b, s_t, d_t)
        for b in range(NB):
            if b + 1 < NB:
                xts[b + 1] = emit_load(b + 1)
            s_t, d_t = emit_stage1(xts.pop(b))
            if pending is not None:
                emit_stage2(*pending)
            pending = (b, s_t, d_t)
        emit_stage2(*pending)
    nc.compile()
    return nc


def _get_nc():
    if "nc" not in _CACHED:
        _CACHED["nc"] = _build()
    return _CACHED["nc"]


def _run(x, **kwargs):
    x = np.ascontiguousarray(np.asarray(x), dtype=np.float32)
    assert x.shape == (N_CORES, C, H, W), x.shape
    nc = _get_nc()
    in_maps = [{"x": np.ascontiguousarray(x[i])} for i in range(N_CORES)]
    res = run_bass_kernel_spmd(nc, in_maps, core_ids=list(range(N_CORES)), **kwargs)
    out = np.stack([res.results[i]["out"] for i in range(N_CORES)], axis=0)
    return out, res


def kernel(x):
    return _run(x)[0]
